# revision 20
# baseline (speedup 1.0000x reference)
"""CTC loss kernel for Trainium2, data-parallel over batch across 8 NeuronCores.

Problem: pred [64, 64, 6736] f32 logits, gt [64, 16] int labels (< blank).
loss = mean_n( -log p_ctc(gt_n | log_softmax(pred_n)) / S ).

Per-core plan (8 examples/core):
  - Stream pred shard (13.8 MB) through SBUF in [128 = 8 ex x 16 t, C] chunks.
    ACT exp + accum_out gives s[n,t] = sum_c exp(pred[n,t,c]) without
    materializing log_softmax (no max subtraction needed: |logits| ~ 5).
  - Gather the 16 target-label logits per (n, t) with per-partition
    indirect DMAs (HW semantics: one offset per partition, consecutive row;
    we use rows of one element, one call per (t-block, label j)).  The
    blank column is a static strided DMA.
  - CTC forward DP in *unnormalized* prob domain on DVE: alphaU *= u_t where
    u_t = exp(pred_ext).  Rescale every few steps (sum / reciprocal / mul),
    collecting the rescale factors; logs deferred to one ACT Ln pass at the
    end so the ACT exp/ln tables swap only once.
  - log p = ln(alphaU[L-1]+alphaU[L-2]) + sum(ln C_k) - sum_t ln s[n,t].
  - Core writes per-example nll/S ([8,1]); host concatenates and means.

Hardware quirks handled here:
  - any instruction (DMA, ACT, ...) fits only ONE sync-wait command: keep
    HWDGE DMA count <= 8 (fresh completion-sem lane each), give consecutive
    ACT ops disjoint tiles, funnel all SWDGE (Pool-queue) DMAs onto a single
    DMASW semaphore (_unify_swdge_lane), and split any remaining multi-wait
    instruction into single-wait no-ops (_split_multi_waits).
"""

import os

import numpy as np

# Persistent XLA compilation cache: makes repeat kernel() calls skip the
# multi-minute neuronx-cc compile when the program is unchanged.
os.environ.setdefault("JAX_COMPILATION_CACHE_DIR", "/tmp/jax_comp_cache")

import concourse.bass as bass
import concourse.mybir as mybir
import concourse.tile as tile
from concourse.bass_utils import run_bass_kernel_spmd

F32 = mybir.dt.float32
I32 = mybir.dt.int32
AF = mybir.ActivationFunctionType
ALU = mybir.AluOpType

# Problem constants
N, T, C, S = 64, 64, 6736, 16
BLANK = C - 1
NCORES = 8
NL = N // NCORES            # examples per core
L = 2 * S + 1               # 33 extended labels
LP = L + 1                  # padded to 34 (dummy slot stays 0)
NTB = 4                     # t-blocks of 16 timesteps -> 128 partitions
TB = T // NTB
CH = C // 2
RENORM_EVERY = 6
RENORM_TS = [t for t in range(1, T) if t % RENORM_EVERY == 0]
NRE = len(RENORM_TS)        # 10

NOFF = NTB * S              # 64 offset columns (tb-major, then j)
AUXW = NOFF + LP            # + 34 bitcast mask columns (rows 0..NL)


def build_bass():
    nc = bass.Bass()
    pred = nc.dram_tensor("pred", [NL, T, C], F32, kind="ExternalInput")
    aux = nc.dram_tensor("aux", [128, AUXW], I32, kind="ExternalInput")
    out = nc.dram_tensor("out", [NL, 1], F32, kind="ExternalOutput")

    pred_flat = pred[:].rearrange("n t c -> (n t c)").unsqueeze(-1)

    with tile.TileContext(nc) as tc:
        with (
            tc.tile_pool(name="stream", bufs=1) as bp,
            tc.tile_pool(name="small", bufs=1) as sp,
        ):
            aux_t = sp.tile([128, AUXW], I32)
            nc.sync.dma_start(out=aux_t[:], in_=aux[:])
            mask_t = aux_t[0:NL, NOFF:AUXW].bitcast(F32)

            # blank column: static strided DMA + exp
            blankT = sp.tile([NL, T], F32)
            nc.sync.dma_start(out=blankT[:], in_=pred[:, :, BLANK : BLANK + 1])
            blankE = sp.tile([NL, T], F32)
            nc.scalar.activation(blankE[:], blankT[:], AF.Exp)

            # u[n, t, l]: even l -> blank, odd l < 2S -> target, l=LP-1 -> 0
            u = sp.tile([NL, T * LP], F32)
            u3 = u[:].rearrange("n (t l) -> n t l", l=LP)
            nc.vector.memset(u3[:, :, LP - 1], 0.0)
            blank_bcast = bass.AP(
                blankE.tensor, blankE[:].offset, [blankE[:].ap[0], [1, T], [0, S + 1]]
            )
            nc.vector.tensor_copy(out=u3[:, :, 0 : 2 * S + 1 : 2], in_=blank_bcast)

            # per-t-block: 16 single-element-row gathers + regroup + exp + copy
            ug_blocks = []
            for tb in range(NTB):
                pg2 = sp.tile([128, S], F32, tag=f"pg2_{tb}")
                for j in range(S):
                    col = tb * S + j
                    nc.gpsimd.indirect_dma_start(
                        out=pg2[:, j : j + 1],
                        out_offset=None,
                        in_=pred_flat,
                        in_offset=bass.IndirectOffsetOnAxis(
                            ap=aux_t[:, col : col + 1], axis=0
                        ),
                    )
                pg3 = sp.tile([NL, TB * S], F32, tag=f"pg3_{tb}")
                # [128=(n,tt), j] -> [n, (tt, j)]
                nc.gpsimd.dma_start(
                    out=pg3[:].rearrange("n (tt j) -> n tt j", j=S), in_=pg2[:]
                )
                ug = sp.tile([NL, TB * S], F32, tag=f"ug_{tb}")
                nc.scalar.activation(ug[:], pg3[:], AF.Exp)
                ug_blocks.append(ug)

            # ---------------- streaming exp-sum over C ---------------------
            chunks = [(k, 0, C) for k in range(NTB - 1)]
            chunks += [(NTB - 1, 0, CH), (NTB - 1, CH, CH)]
            hparts = {}
            for ci, (k, c0, cw) in enumerate(chunks):
                bt = bp.tile([128, cw], F32, tag=f"stream{ci}")
                src = pred[:, k * TB : (k + 1) * TB, c0 : c0 + cw]
                nc.sync.dma_start(out=bt[:], in_=src)
                hk = sp.tile([128, 1], F32, tag=f"h{ci}")
                nc.scalar.activation(bt[:], bt[:], AF.Exp, accum_out=hk[:])
                hparts.setdefault(k, []).append(hk)
            stile = sp.tile([128, NTB], F32)
            for k in range(NTB):
                hs = hparts[k]
                if len(hs) == 1:
                    nc.vector.tensor_copy(out=stile[:, k : k + 1], in_=hs[0][:])
                else:
                    nc.vector.tensor_add(
                        out=stile[:, k : k + 1], in0=hs[0][:], in1=hs[1][:]
                    )

            # ---------------- CTC forward DP (DVE) -------------------------
            buf = sp.tile([NL, LP + 2], F32)  # cols 0,1 guard zeros; 2.. = alpha
            nc.vector.memset(buf[:], 0.0)

            tmp1 = sp.tile([NL, LP], F32)
            tmp2 = sp.tile([NL, LP], F32)
            tmp3 = sp.tile([NL, LP], F32)
            rlog = sp.tile([NL, NRE + 1], F32)
            rinv = sp.tile([NL, 1], F32)

            a = buf[:, 2 : LP + 2]
            a1 = buf[:, 1 : LP + 1]
            a2 = buf[:, 0:LP]
            for tb in range(NTB):
                # copy this block's targets into u, then run its DP steps
                ug3 = ug_blocks[tb][:].rearrange("n (tt j) -> n tt j", j=S)
                nc.vector.tensor_copy(
                    out=u3[:, tb * TB : (tb + 1) * TB, 1 : 2 * S : 2], in_=ug3
                )
                if tb == 0:
                    nc.vector.tensor_copy(out=buf[:, 2:4], in_=u3[:, 0, 0:2])
                t0 = max(1, tb * TB)
                for t in range(t0, (tb + 1) * TB):
                    ut = u3[:, t, :]
                    nc.vector.tensor_add(out=tmp1[:], in0=a, in1=a1)
                    nc.vector.tensor_mul(out=tmp2[:], in0=a2, in1=mask_t)
                    nc.vector.tensor_add(out=tmp3[:], in0=tmp1[:], in1=tmp2[:])
                    nc.vector.tensor_mul(out=a, in0=tmp3[:], in1=ut)
                    if t in RENORM_TS:
                        k = RENORM_TS.index(t)
                        nc.vector.tensor_reduce(
                            out=rlog[:, k : k + 1],
                            in_=a,
                            axis=mybir.AxisListType.X,
                            op=ALU.add,
                        )
                        nc.vector.reciprocal(out=rinv[:], in_=rlog[:, k : k + 1])
                        nc.vector.tensor_scalar_mul(out=a, in0=a, scalar1=rinv[:])

            # final forward prob: alpha[L-1] + alpha[L-2] (cols L+1, L in buf)
            nc.vector.tensor_add(
                out=rlog[:, NRE : NRE + 1],
                in0=buf[:, L : L + 1],
                in1=buf[:, L + 1 : L + 2],
            )

            # ---------------- logs + assembly ------------------------------
            lnr = sp.tile([NL, NRE + 1], F32)
            nc.scalar.activation(lnr[:], rlog[:], AF.Ln)
            rsum = sp.tile([NL, 1], F32)
            nc.vector.tensor_reduce(
                out=rsum[:], in_=lnr[:], axis=mybir.AxisListType.X, op=ALU.add
            )

            lns = sp.tile([128, NTB], F32)
            nc.scalar.activation(lns[:], stile[:], AF.Ln)
            sb = sp.tile([128, 1], F32)
            nc.vector.tensor_reduce(
                out=sb[:], in_=lns[:], axis=mybir.AxisListType.X, op=ALU.add
            )
            # regroup partitions (n*TB + tt) -> [NL, TB] and finish the sum
            zt = sp.tile([NL, TB], F32)
            nc.gpsimd.dma_start(out=zt[:], in_=sb[:])
            zs = sp.tile([NL, 1], F32)
            nc.vector.tensor_reduce(
                out=zs[:], in_=zt[:], axis=mybir.AxisListType.X, op=ALU.add
            )

            # nll/S = (sum_t ln s - (ln fwd + sum ln C_k)) / S
            res = sp.tile([NL, 1], F32)
            nc.vector.tensor_tensor(
                out=res[:], in0=zs[:], in1=rsum[:], op=ALU.subtract
            )
            res2 = sp.tile([NL, 1], F32)
            nc.vector.tensor_scalar_mul(out=res2[:], in0=res[:], scalar1=1.0 / S)
            nc.sync.dma_start(out=out[:], in_=res2[:])

    return nc


def _unify_swdge_lane(nc):
    """Funnel every Pool-queue (SWDGE) DMA onto one DMASW semaphore.

    Tile round-robins SWDGE completions over 8 DMASW lanes; a consumer of
    tiles written by many SWDGE DMAs then needs one wait per lane, but the
    hardware encodings fit a single wait.  The Pool queue executes its DMAs
    FIFO, so rebasing every update onto lane 0 with cumulative values and
    remapping (lane, value) waits to the corresponding cumulative value is
    equivalent (and collapses multi-lane waits to one max)."""
    swdge = []
    for bb in nc.main_func.blocks:
        for inst in bb.instructions:
            if (
                isinstance(inst, mybir.InstDMACopy)
                and getattr(inst, "queue", None) == "qPoolDynamic"
            ):
                swdge.append(inst)
    if not swdge:
        return
    # unified sem: first SWDGE DMA's DMASW update
    def dmasw_update(inst):
        si = inst.sync_info
        if si is None:
            return None
        for up in si.on_update:
            if up.ant_name and up.ant_name.startswith("DMASW"):
                return up
        return None

    uni = dmasw_update(swdge[0])
    assert uni is not None, "first SWDGE DMA has no DMASW update"
    uni_id, uni_name = uni.id, uni.ant_name

    # map (lane_name, seq_index) -> global order index
    lane_seq = {}
    gidx = {}
    for g, inst in enumerate(swdge):
        up = dmasw_update(inst)
        assert up is not None, f"SWDGE DMA {inst.name} missing DMASW update"
        lane = up.ant_name
        lane_seq.setdefault(lane, []).append(g)
        gidx[inst.name] = g
        # rebase update onto the unified lane
        others = [u for u in inst.sync_info.on_update if u is not up]
        newup = mybir.SyncUpdate(
            sync_type="semaphore",
            id=uni_id,
            ant_name=uni_name,
            update_mode="sem-add-imm",
            update_value=16,
        )
        inst.sync_info = mybir.SyncInfo(
            on_wait=list(inst.sync_info.on_wait), on_update=others + [newup]
        )

    # remap all waits on any DMASW lane
    for bb in nc.main_func.blocks:
        for inst in bb.instructions:
            si = getattr(inst, "sync_info", None)
            if si is None or not si.on_wait:
                continue
            new_waits = []
            uni_val = None
            changed = False
            for w in si.on_wait:
                if w.ant_name and w.ant_name.startswith("DMASW"):
                    changed = True
                    m = w.wait_value // 16  # m-th completion on that lane
                    assert w.wait_value % 16 == 0 and m >= 1, w
                    g = lane_seq[w.ant_name][m - 1]
                    v = 16 * (g + 1)
                    uni_val = v if uni_val is None else max(uni_val, v)
                else:
                    new_waits.append(w)
            if not changed:
                continue
            if uni_val is not None:
                new_waits.append(
                    mybir.SyncWait(
                        sync_type="semaphore",
                        id=uni_id,
                        ant_name=uni_name,
                        wait_mode="sem-ge-imm",
                        wait_value=uni_val,
                    )
                )
            inst.sync_info = mybir.SyncInfo(
                on_wait=new_waits, on_update=list(si.on_update)
            )


def _split_multi_waits(nc, maxw=1):
    """This compiler's codegen rejects >1 sync-wait command per instruction
    (setupSyncWait 'Too many sync wait commands').  Tile's kernel-tail drain
    aggregates one wait per live semaphore; split the excess into a chain of
    single-wait no-ops on the same engine right before the instruction."""
    for bb in nc.main_func.blocks:
        heavy = [
            (i, inst)
            for i, inst in enumerate(bb.instructions)
            if getattr(inst, "sync_info", None) is not None
            and inst.sync_info.on_wait
            and len(inst.sync_info.on_wait) > maxw
        ]
        for pos, inst in reversed(heavy):
            waits = list(inst.sync_info.on_wait)
            keep, extra = waits[:maxw], waits[maxw:]
            inst.sync_info = mybir.SyncInfo(
                on_wait=keep, on_update=list(inst.sync_info.on_update)
            )
            for j, w in enumerate(reversed(extra)):
                nop = mybir.InstNoOp(
                    name=f"{inst.name}-waitsplit-{j}",
                    ins=[],
                    outs=[],
                    sync_info=mybir.SyncInfo(on_wait=[w], on_update=[]),
                )
                nop.engine = inst.engine
                bb.instructions.insert(pos, nop)


def prepare_hw(nc):
    _unify_swdge_lane(nc)
    _split_multi_waits(nc)
    return nc


def make_core_inputs(pred_full, gt_full, core):
    nsl = slice(core * NL, (core + 1) * NL)
    predc = np.ascontiguousarray(pred_full[nsl], dtype=np.float32)
    gtc = np.asarray(gt_full[nsl]).astype(np.int64)

    # offsets: col = tb*S + j; partition p = (n, tt):
    #   flat elem index of pred[n, 16*tb + tt, gt[n, j]]
    aux = np.zeros((128, AUXW), np.int32)
    p_n = np.arange(128) // TB
    p_tt = np.arange(128) % TB
    for tb in range(NTB):
        t_abs = tb * TB + p_tt  # [128]
        base = (p_n * T + t_abs) * C
        for j in range(S):
            aux[:, tb * S + j] = base + gtc[p_n, j]

    m = np.zeros((NL, LP), np.float32)
    m[:, 1] = 1.0
    for j in range(1, S):
        m[:, 2 * j + 1] = (gtc[:, j] != gtc[:, j - 1]).astype(np.float32)
    aux[0:NL, NOFF:AUXW] = m.view(np.int32)

    return {"pred": predc, "aux": aux}


_NC_CACHE = {}


def kernel(pred, gt):
    in_maps = [make_core_inputs(pred, gt, c) for c in range(NCORES)]
    if "nc" not in _NC_CACHE:
        _NC_CACHE["nc"] = prepare_hw(build_bass())
    nc = _NC_CACHE["nc"]
    res = run_bass_kernel_spmd(nc, in_maps, core_ids=list(range(NCORES)))
    _NC_CACHE["last_results"] = res
    vals = np.concatenate([r["out"][:, 0] for r in res.results])
    return np.array(vals.mean(), dtype=np.float32)


if __name__ == "__main__":
    rng = np.random.default_rng(0)
    pred = rng.standard_normal((N, T, C), dtype=np.float32)
    gt = rng.integers(0, BLANK, size=(N, S)).astype(np.int32)
    print(kernel(pred=pred, gt=gt))
